# revision 18
# baseline (speedup 1.0000x reference)
"""Grouped-query attention (B=2, S=2048, D=1024, 16 q heads / 4 kv heads,
RoPE, softmax, out-proj) on 8 Trainium2 NeuronCores.

The axon tunnel is ~50 MB/s up / ~40 MB/s down, FULL-DUPLEX, with ~45-90 ms
per-op latency; async-issued executions pipeline at ~5 ms marginal cost.
The design minimizes bytes and overlaps the three serial phases:

  * core c = (b, g): b = c // 4 (batch), g = c % 4 (512-query sequence
    block).  Every core computes ALL 16 heads for its queries, so each
    core's inputs and outputs are DISJOINT slices -- no replication.
  * K/V projections (1024 -> 256 dims) + K RoPE run on the host in f32
    BLAS (~40 ms) so only the small projected K/V go over the wire
    (K^T bf16 2 MB, V int8 + per-row scales 1 MB).
  * a 6-call PIPELINE per invocation:
      1. "gather" programs (one for K^T bf16, one for V int8): upload a
         slice, AllGather collective across each 4-core batch group,
         output full K/V as device-resident arrays (never fetched).
         K is dispatched before V/q host prep so its upload streams early.
      2. 4x "attend" program: each takes a 128-query chunk (int8 + per-row
         scales, 1.05 MB) + the gathered K/V arrays, dequantizes V on
         device, computes attention + out-projection, returns an int8 +
         scales chunk (1.05 MB).
    Later-stage uploads, all execs, and all fetches overlap via async
    dispatch + full-duplex transfers; D2H copies are PRE-POSTED with
    copy_to_host_async so chunks stream back the moment they complete;
    fetch/decode runs in threads.
  * weights / tables / zero-output buffers are device-resident jax arrays
    cached across calls; both jit'd dispatch fns are built once.
"""

import os
import sys
import zlib

import numpy as np

for _p in ("/opt/trn_rl_repo", "/root/.axon_site/_ro/trn_rl_repo"):
    if os.path.isdir(_p) and _p not in sys.path:
        sys.path.append(_p)

B, S, D = 2, 2048, 1024
NHEAD, NUM_KV, DK = 16, 4, 64
SL = 512                          # queries per core
NCH = 4                           # pipeline chunks per core
CL = SL // NCH                    # 128 queries per chunk
NCORES = 8
P = 128
NT = S // P                       # 16 t-tiles of 128
SCALE = 1.0 / float(np.sqrt(DK))
ROPE_BASE = 10000.0

QB = CL * D + CL * 4              # int8 q chunk + f32 scales, per core
OB = CL * D + CL * 4              # int8 out chunk + f32 scales, per core
VB = SL * 256 + SL * 4            # int8 v slice + f32 scales, per core

_CACHE: dict = {}
_RUNNER = None


def _host_tables():
    """cos/sin [S, 64] with the rotate-half convention of the reference."""
    if "tables" in _CACHE:
        return _CACHE["tables"]
    inv_freq = 1.0 / (ROPE_BASE ** (np.arange(0, DK, 2, dtype=np.float64) / DK))
    t = np.arange(S, dtype=np.float64)
    freqs = np.outer(t, inv_freq)                       # [S, 32]
    emb = np.concatenate([freqs, freqs], axis=-1)       # [S, 64]
    _CACHE["tables"] = (np.cos(emb).astype(np.float32),
                        np.sin(emb).astype(np.float32))
    return _CACHE["tables"]


def _perm_np():
    perm = np.zeros((P, P), dtype=np.float32)
    for blk in (0, DK):
        for q in range(32):
            perm[blk + q + 32, blk + q] = -1.0          # rot[q] = -x[q+32]
        for q in range(32, DK):
            perm[blk + q - 32, blk + q] = 1.0           # rot[q] = x[q-32]
    return perm


def _emit_gather(tc, aps, nelem, dt):
    """AllGather one flat [nelem] slice into [4*nelem] (device-kept)."""
    import concourse.mybir as mybir

    nc = tc.nc
    groups = [[0, 1, 2, 3], [4, 5, 6, 7]]
    from contextlib import ExitStack
    ctx = ExitStack()
    dram = ctx.enter_context(tc.tile_pool(name="dram", bufs=1, space="DRAM"))
    gb = dram.tile([nelem], dt, tag="gb", name="gb")
    gg = dram.tile([4 * nelem], dt, tag="gg", name="gg")
    nc.gpsimd.dma_start(gb[:], aps["g_in"][:])
    nc.gpsimd.collective_compute(
        "AllGather", mybir.AluOpType.bypass, replica_groups=groups,
        ins=[gb.opt()], outs=[gg.opt()])
    nc.gpsimd.dma_start(aps["g_out"][:], gg[:])
    ctx.close()


def _emit_attend(tc, aps):
    """Attention for one 128-query chunk from pre-gathered K/V."""
    import concourse.bass as bass
    import concourse.mybir as mybir

    nc = tc.nc
    f32 = mybir.dt.float32
    bf16 = mybir.dt.bfloat16
    int8 = mybir.dt.int8
    AF = mybir.ActivationFunctionType
    AX = mybir.AxisListType

    kg_in, vg_in = aps["kg_in"], aps["vg_in"]
    wq_t, wo_t = aps["wq_t"], aps["wo_t"]

    from contextlib import ExitStack
    ctx = ExitStack()
    const = ctx.enter_context(tc.tile_pool(name="const", bufs=1))
    persist = ctx.enter_context(tc.tile_pool(name="persist", bufs=1))
    work = ctx.enter_context(tc.tile_pool(name="work", bufs=3))
    ptpool = ctx.enter_context(tc.tile_pool(name="ptp", bufs=2))
    psA = ctx.enter_context(
        tc.tile_pool(name="psA", bufs=4, space=bass.MemorySpace.PSUM))
    psT = ctx.enter_context(
        tc.tile_pool(name="psT", bufs=2, space=bass.MemorySpace.PSUM))

    def psa(name):
        return psA.tile([P, 512], f32, tag="ps", name=name)

    # ---- constants (device-resident across calls) ------------------------
    wq_sb = const.tile([P, 8 * D], bf16, tag="wq", name="wq_sb")
    nc.sync.dma_start(
        wq_sb.rearrange("p (k m) -> p k m", k=8),
        wq_t.rearrange("(k p) m -> p k m", p=P),
    )
    wo_sb = const.tile([P, 8 * D], bf16, tag="wo", name="wo_sb")
    nc.sync.dma_start(
        wo_sb.rearrange("p (k m) -> p k m", k=8),
        wo_t.rearrange("(k p) m -> p k m", p=P),
    )
    cos_sb = const.tile([P, CL], f32, tag="cos", name="cos_sb")
    nc.sync.dma_start(cos_sb[:], aps["cos_t"][:])
    sin_sb = const.tile([P, CL], f32, tag="sin", name="sin_sb")
    nc.sync.dma_start(sin_sb[:], aps["sin_t"][:])
    perm_sb = const.tile([P, P], f32, tag="perm", name="perm_sb")
    nc.sync.dma_start(perm_sb[:], aps["perm"][:])
    id_f = const.tile([P, P], f32, tag="idf", name="id_f")
    nc.sync.dma_start(id_f[:], aps["ident"][:])
    bq_sb = const.tile([P, 8], f32, tag="bq", name="bq_sb")
    nc.sync.dma_start(bq_sb[:], aps["bq_c"][:])
    bo_sb = const.tile([P, 8], f32, tag="bo", name="bo_sb")
    nc.sync.dma_start(bo_sb[:], aps["bo_c"][:])
    id_b = const.tile([P, P], bf16, tag="idb", name="id_b")
    nc.vector.tensor_copy(id_b[:], id_f[:])

    # ---- K/V from pre-gathered device arrays -----------------------------
    # kg_in rows: pe*256 + kv*64 + c  (t = pe*512 + t_loc)
    kT_sb = [persist.tile([P, S], bf16, tag=f"kT{kv}", name=f"kT_sb{kv}")
             for kv in range(NUM_KV)]
    for kv in range(NUM_KV):
        for half in range(2):
            for pe in range(4):
                r0 = (pe * 256 + kv * DK) * SL
                nc.sync.dma_start(
                    kT_sb[kv][half * DK:(half + 1) * DK,
                              pe * SL:(pe + 1) * SL],
                    kg_in[r0:r0 + DK * SL].rearrange("(c t) -> c t", c=DK))
    # v_aug[kv]: [128, 16 t-tiles x 65] bf16 (V columns + ones column)
    vA_sb = [persist.tile([P, NT * (DK + 1)], bf16, tag=f"vA{kv}",
                          name=f"vA_sb{kv}")
             for kv in range(NUM_KV)]
    # per-t dequant scales: vsc[p, tt] f32, t = tt*128 + p, tt = pe*4 + q
    vsc = persist.tile([P, NT], f32, tag="vsc", name="vsc")
    for pe in range(4):
        nc.sync.dma_start(
            vsc.bitcast(int8)[:, pe * 16:(pe + 1) * 16].rearrange(
                "p (q b) -> p q b", q=4),
            vg_in[pe * VB + SL * 256:pe * VB + VB].rearrange(
                "(q p b) -> p q b", q=4, p=P))
    for kv in range(NUM_KV):
        for tt in range(NT):
            pe, qq = tt // 4, tt % 4
            base = pe * VB + qq * P * 256
            vi8 = work.tile([P, DK], int8, tag="vi8", name=f"vi8_{kv}_{tt}")
            nc.sync.dma_start(
                vi8[:],
                vg_in[base:base + P * 256].rearrange(
                    "(p c) -> p c", p=P)[:, kv * DK:(kv + 1) * DK])
            nc.scalar.activation(
                vA_sb[kv][:, tt * (DK + 1):tt * (DK + 1) + DK],
                vi8[:], AF.Copy, scale=vsc[:, tt:tt + 1])
        nc.vector.memset(
            vA_sb[kv].rearrange("p (tt e) -> p tt e", e=DK + 1)
            [:, :, DK:DK + 1], 1.0)

    # ---- q chunk: int8 + scales -> dequant -> transpose ------------------
    q_in = aps["q_in"]                      # flat [CL*D + CL*4] int8
    qsc = persist.tile([P, 1], f32, tag="qsc", name="qsc")
    nc.sync.dma_start(
        qsc.bitcast(int8),
        q_in[CL * D:CL * D + CL * 4].rearrange("(p b) -> p b", p=P))
    qi8 = persist.tile([P, D], int8, tag="qi8", name="qi8")
    nc.sync.dma_start(qi8[:], q_in[0:CL * D].rearrange("(p d) -> p d", p=P))
    qnat = persist.tile([P, D], bf16, tag="qnat", name="qnat")
    nc.scalar.activation(qnat[:], qi8[:], AF.Copy, scale=qsc[:, 0:1])

    qT_sb = persist.tile([P, 8 * CL], bf16, tag="qT", name="qT_sb")
    for kd in range(8):
        tp = psT.tile([P, P], bf16, tag="tp", name=f"tq{kd}")
        nc.tensor.transpose(tp[:], qnat[:, kd * P:(kd + 1) * P], id_b[:])
        nc.vector.tensor_copy(qT_sb[:, kd * CL:(kd + 1) * CL], tp[:])

    # ---- Q projection + rope ---------------------------------------------
    qs_sb = persist.tile([P, 8 * CL], bf16, tag="qs", name="qs_sb")
    for m in range(8):
        ps = psa(f"psQ{m}")
        for kd in range(8):
            nc.tensor.matmul(ps[:, 0:CL],
                             wq_sb[:, kd * D + m * P:kd * D + (m + 1) * P],
                             qT_sb[:, kd * CL:(kd + 1) * CL],
                             start=(kd == 0), stop=(kd == 7))
        qraw = work.tile([P, CL], f32, tag="qraw", name=f"qraw{m}")
        nc.vector.tensor_scalar_add(qraw[:], ps[:, 0:CL], bq_sb[:, m:m + 1])
        sh = psa(f"shq{m}")
        nc.tensor.matmul(sh[:, 0:CL], perm_sb[:], qraw[:],
                         start=True, stop=True)
        tmp = work.tile([P, CL], f32, tag="rtmp", name=f"rtmp{m}")
        nc.vector.tensor_mul(tmp[:], sh[:, 0:CL], sin_sb[:])
        nc.vector.tensor_mul(qraw[:], qraw[:], cos_sb[:])
        nc.vector.tensor_add(qs_sb[:, m * CL:(m + 1) * CL], qraw[:], tmp[:])

    # ---- attention: 16 heads, head h -> q tile h//2 base (h%2)*64 --------
    ctxT2 = persist.tile([P, 8 * CL], bf16, tag="ctxT2", name="ctxT2")
    for pr in range(8):                     # head pair -> 128 ctx dims
        ctxp = work.tile([P, P], f32, tag="ctxp", name=f"ctxp{pr}")
        for hh in range(2):
            h = 2 * pr + hh
            m, pb, kv = h // 2, (h % 2) * DK, h // 4
            pt = ptpool.tile([P, NT * CL], bf16, tag="pt", name=f"pt{h}")
            for tt in range(NT):
                sc = psa(f"sc{h}_{tt}")
                nc.tensor.matmul(sc[:, 0:CL],
                                 kT_sb[kv][pb:pb + DK, tt * P:(tt + 1) * P],
                                 qs_sb[pb:pb + DK, m * CL:(m + 1) * CL],
                                 start=True, stop=True)
                nc.scalar.activation(pt[:, tt * CL:(tt + 1) * CL],
                                     sc[:, 0:CL], AF.Exp, scale=SCALE)
            pv = psa(f"pv{h}")
            for tt in range(NT):
                nc.tensor.matmul(
                    pv[:, 0:DK + 1],
                    pt[:, tt * CL:(tt + 1) * CL],
                    vA_sb[kv][:, tt * (DK + 1):(tt + 1) * (DK + 1)],
                    start=(tt == 0), stop=(tt == NT - 1))
            rec = work.tile([P, 1], f32, tag="rec", name=f"rec{h}")
            nc.vector.reciprocal(rec[:], pv[:, DK:DK + 1])
            nc.vector.tensor_scalar_mul(
                ctxp[:, hh * DK:hh * DK + DK], pv[:, 0:DK], rec[:, 0:1])
        tf = psT.tile([P, P], f32, tag="tp", name=f"tc{pr}")
        nc.tensor.transpose(tf[:], ctxp[:], id_f[:])
        nc.vector.tensor_copy(ctxT2[:, pr * CL:(pr + 1) * CL], tf[:])

    # ---- out projection + transpose + int8 quantize ----------------------
    onat = persist.tile([P, D], bf16, tag="onat", name="onat")
    for nk in range(8):
        po = psa(f"po{nk}")
        for pr in range(8):
            nc.tensor.matmul(po[:, 0:CL],
                             wo_sb[:, pr * D + nk * P:pr * D + (nk + 1) * P],
                             ctxT2[:, pr * CL:(pr + 1) * CL],
                             start=(pr == 0), stop=(pr == 7))
        osb = work.tile([P, CL], bf16, tag="osb", name=f"osb{nk}")
        nc.vector.tensor_scalar_add(osb[:], po[:, 0:CL], bo_sb[:, nk:nk + 1])
        tb = psT.tile([P, P], bf16, tag="tp", name=f"to{nk}")
        nc.tensor.transpose(tb[:], osb[:], id_b[:])
        nc.vector.tensor_copy(onat[:, nk * P:(nk + 1) * P], tb[:])

    out_h = aps["out_c"]                    # flat [CL*D + CL*4] int8
    m_ = work.tile([P, 1], f32, tag="omax", name="omax")
    nc.vector.tensor_reduce(m_[:], onat[:], AX.X, mybir.AluOpType.max,
                            apply_absolute_value=True)
    osc = work.tile([P, 1], f32, tag="osc", name="osc")
    nc.vector.tensor_scalar_mul(osc[:], m_[:], 1.0 / 127.0)
    rcp = work.tile([P, 1], f32, tag="orcp", name="orcp")
    nc.vector.reciprocal(rcp[:], m_[:])
    nc.vector.tensor_scalar_mul(rcp[:], rcp[:], 127.0)
    oq = work.tile([P, D], int8, tag="oq", name="oq")
    nc.scalar.activation(oq[:], onat[:], AF.Copy, scale=rcp[:, 0:1])
    nc.sync.dma_start(
        out_h[0:CL * D].rearrange("(p d) -> p d", p=P), oq[:])
    nc.sync.dma_start(
        out_h[CL * D:CL * D + CL * 4].rearrange("(p b) -> p b", p=P),
        osc.bitcast(int8))

    ctx.close()


def build_module(role):
    """Build + compile one SPMD program ('gather' or 'attend')."""
    if role in _CACHE:
        return _CACHE[role]
    from concourse import bacc, mybir
    import concourse.tile as tile

    nc = bacc.Bacc("TRN2", target_bir_lowering=False, debug=False,
                   enable_asserts=False, num_devices=NCORES)
    f32 = mybir.dt.float32
    bf16 = mybir.dt.bfloat16
    int8 = mybir.dt.int8
    if role == "gather":
        shapes = {"g_in": ((256 * SL,), bf16)}
        outs = {"g_out": ((4 * 256 * SL,), bf16)}
    elif role == "gatherv":
        shapes = {"g_in": ((VB,), int8)}
        outs = {"g_out": ((4 * VB,), int8)}
    else:
        shapes = {
            "q_in": ((QB,), int8),
            "kg_in": ((4 * 256 * SL,), bf16),
            "vg_in": ((4 * VB,), int8),
            "wq_t": ((D, D), bf16),
            "wo_t": ((D, D), bf16),
            "cos_t": ((P, CL), f32),
            "sin_t": ((P, CL), f32),
            "perm": ((P, P), f32),
            "ident": ((P, P), f32),
            "bq_c": ((P, 8), f32),
            "bo_c": ((P, 8), f32),
        }
        outs = {"out_c": ((OB,), int8)}
    aps = {name: nc.dram_tensor(name, list(shp), dt, kind="ExternalInput").ap()
           for name, (shp, dt) in shapes.items()}
    for name, (shp, dt) in outs.items():
        aps[name] = nc.dram_tensor(name, list(shp), dt,
                                   kind="ExternalOutput").ap()
    with tile.TileContext(nc) as tc:
        if role == "gather":
            _emit_gather(tc, aps, 256 * SL, bf16)
        elif role == "gatherv":
            _emit_gather(tc, aps, VB, int8)
        else:
            _emit_attend(tc, aps)
    nc.compile()
    _CACHE[role] = nc
    return nc


class _Runner:
    """Caches both jit'd dispatch fns + device-resident constants."""

    def __init__(self):
        import jax
        from jax.sharding import Mesh, PartitionSpec, NamedSharding

        self.jax = jax
        devices = jax.devices()[:NCORES]
        self.mesh = Mesh(np.asarray(devices), ("core",))
        self.sharding = NamedSharding(self.mesh, PartitionSpec("core"))
        self.fn_gather, self.meta_g = self._make_fn("gather")
        self.fn_gatherv, self.meta_gv = self._make_fn("gatherv")
        self.fn_attend, self.meta_a = self._make_fn("attend")
        self.const_dev = None
        self.const_key = None
        self.pool = None

    def _make_fn(self, role):
        import jax
        import concourse.mybir as mybir
        from concourse import bass2jax
        from concourse.bass_interp import get_hw_module
        from jax.sharding import PartitionSpec
        from jax.experimental.shard_map import shard_map

        nc = build_module(role)
        nc.m = get_hw_module(nc.m)
        part_name = (nc.partition_id_tensor.name
                     if nc.partition_id_tensor else None)
        in_names, out_names, out_avals = [], [], []
        for alloc in nc.m.functions[0].allocations:
            if not isinstance(alloc, mybir.MemoryLocationSet):
                continue
            name = alloc.memorylocations[0].name
            if alloc.kind == "ExternalInput":
                if name != part_name:
                    in_names.append(name)
            elif alloc.kind == "ExternalOutput":
                out_names.append(name)
                out_avals.append(jax.core.ShapedArray(
                    tuple(alloc.tensor_shape), mybir.dt.np(alloc.dtype)))
        all_in = tuple(in_names) + tuple(out_names) + (
            (part_name,) if part_name else ())

        def _body(*args):
            operands = list(args)
            if part_name is not None:
                operands.append(bass2jax.partition_id_tensor())
            return tuple(bass2jax._bass_exec_p.bind(
                *operands, out_avals=tuple(out_avals), in_names=all_in,
                out_names=tuple(out_names), lowering_input_output_aliases=(),
                sim_require_finite=True, sim_require_nnan=True, nc=nc))

        nio = len(in_names) + len(out_names)
        fn = jax.jit(
            shard_map(_body, mesh=self.mesh,
                      in_specs=(PartitionSpec("core"),) * nio,
                      out_specs=(PartitionSpec("core"),) * len(out_names),
                      check_rep=False),
            keep_unused=True)
        return fn, {"in_names": in_names, "out_names": out_names,
                    "out_avals": out_avals}

    def _const_args(self, inputs):
        """Device-resident per-core constants, rebuilt only if weights change."""
        import ml_dtypes
        bf16 = ml_dtypes.bfloat16
        f = np.float32
        Wq, Wo = np.asarray(inputs["Wq"], f), np.asarray(inputs["Wo"], f)
        bq, bo = np.asarray(inputs["bq"], f), np.asarray(inputs["bo"], f)
        key = zlib.crc32(Wq.tobytes()) ^ zlib.crc32(Wo.tobytes()) ^ \
            zlib.crc32(bq.tobytes()) ^ zlib.crc32(bo.tobytes())
        if self.const_dev is not None and key == self.const_key:
            return self.const_dev

        cos, sin = _host_tables()                      # [S, 64]
        consts = {}
        consts["wq_t"] = np.tile(Wq.T.astype(bf16), (NCORES, 1))
        consts["wo_t"] = np.tile(Wo.T.astype(bf16), (NCORES, 1))
        for ch in range(NCH):                          # per-chunk tables
            cos_c, sin_c = [], []
            for c in range(NCORES):
                g = c % 4
                lo = g * SL + ch * CL
                cs = cos[lo:lo + CL, :].T               # [64, 128]
                sn = sin[lo:lo + CL, :].T
                cos_c.append(np.concatenate([cs, cs], axis=0))
                sin_c.append(np.concatenate([sn, sn], axis=0))
            consts[f"cos_t{ch}"] = np.concatenate(cos_c, axis=0).astype(f)
            consts[f"sin_t{ch}"] = np.concatenate(sin_c, axis=0).astype(f)
        consts["perm"] = np.tile(_perm_np(), (NCORES, 1))
        consts["ident"] = np.tile(np.eye(P, dtype=f), (NCORES, 1))
        consts["bq_c"] = np.tile(
            np.ascontiguousarray(bq.reshape(8, P).T), (NCORES, 1))
        consts["bo_c"] = np.tile(
            np.ascontiguousarray(bo.reshape(8, P).T), (NCORES, 1))
        dev = {k: self.jax.device_put(v, self.sharding)
               for k, v in consts.items()}
        for tag, meta in (("g", self.meta_g), ("gv", self.meta_gv),
                          ("a", self.meta_a)):
            dev[f"__zeros_{tag}__"] = [
                self.jax.device_put(
                    np.zeros((NCORES * av.shape[0],) + tuple(av.shape[1:]),
                             av.dtype), self.sharding)
                for av in meta["out_avals"]]
        self.jax.block_until_ready(
            [v for k, v in dev.items() if not k.startswith("__")]
            + dev["__zeros_g__"] + dev["__zeros_gv__"] + dev["__zeros_a__"])
        self.const_dev, self.const_key = dev, key
        return dev

    def __call__(self, inputs):
        import concurrent.futures as cf
        import ml_dtypes
        bf16 = ml_dtypes.bfloat16
        f = np.float32
        if self.pool is None:
            self.pool = cf.ThreadPoolExecutor(NCH + 1)
        query = np.asarray(inputs["query"], f)
        key_ = np.asarray(inputs["key"], f)
        value = np.asarray(inputs["value"], f)
        Wk = np.asarray(inputs["Wk"], f)
        Wv = np.asarray(inputs["Wv"], f)
        bk = np.asarray(inputs["bk"], f)
        bv = np.asarray(inputs["bv"], f)
        cd = self._const_args(inputs)

        def prep_dispatch_k():
            # K prep + gather dispatch: upload starts streaming asap
            K = key_.reshape(-1, D) @ Wk.T + bk        # [B*S, 256]
            cos, sin = _host_tables()                  # [S, 64]
            Kh = K.reshape(B, S, NUM_KV, DK)
            rot = np.concatenate(
                [-Kh[..., DK // 2:], Kh[..., :DK // 2]], axis=-1)
            Kh = Kh * cos[None, :, None, :] + rot * sin[None, :, None, :]
            kT_g = np.ascontiguousarray(
                Kh.reshape(B, 4, SL, NUM_KV * DK).transpose(0, 1, 3, 2)
            ).astype(bf16).reshape(NCORES * 256 * SL)
            return self.fn_gather(kT_g, *cd["__zeros_g__"])[0]

        def prep_dispatch_v():
            V = value.reshape(-1, D) @ Wv.T + bv       # [B*S, 256]
            vamax = np.maximum(np.abs(V).max(axis=1), 1e-30)
            vi8 = np.rint(V * (127.0 / vamax)[:, None]).astype(np.int8)
            vbuf = np.empty((NCORES, VB), np.int8)
            vbuf[:, :SL * 256] = vi8.reshape(NCORES, SL * 256)
            vbuf[:, SL * 256:] = (vamax / 127.0).astype(f).reshape(
                NCORES, SL).view(np.int8)
            return self.fn_gatherv(vbuf.reshape(-1), *cd["__zeros_gv__"])[0]

        kg_fut = self.pool.submit(prep_dispatch_k)
        vg_fut = self.pool.submit(prep_dispatch_v)

        # --- q pack on the main thread (overlaps K/V prep + uploads) -------
        q2 = query.reshape(NCORES * SL, D)
        amax = np.maximum(np.abs(q2).max(axis=1), 1e-30)
        qi8 = np.rint(q2 * (127.0 / amax)[:, None]).astype(np.int8)
        qsc = (amax / 127.0).astype(f)
        qbuf = np.empty((NCORES, NCH, QB), np.int8)
        qbuf[:, :, :CL * D] = qi8.reshape(NCORES, NCH, CL * D)
        qbuf[:, :, CL * D:] = qsc.reshape(NCORES, NCH, CL).view(np.int8)
        kg_dev = kg_fut.result()
        vg_dev = vg_fut.result()

        # --- attention chunks (async, pipelined) ---------------------------
        chunk_outs = []
        for ch in range(NCH):
            a_acts = {"q_in": np.ascontiguousarray(
                          qbuf[:, ch, :]).reshape(-1),
                      "kg_in": kg_dev, "vg_in": vg_dev,
                      "cos_t": cd[f"cos_t{ch}"], "sin_t": cd[f"sin_t{ch}"]}
            a_args = [a_acts.get(n, cd.get(n))
                      for n in self.meta_a["in_names"]]
            a_args.extend(cd["__zeros_a__"])
            o = self.fn_attend(*a_args)[0]
            # pre-post the D2H copy so the terminal streams the chunk back
            # the moment it's ready (skips a ready->request round trip)
            o.copy_to_host_async()
            chunk_outs.append(o)

        # --- fetch + decode in threads as chunks complete ------------------
        out = np.empty((NCORES, NCH, CL, D), f)

        def dec(ch):
            a = np.asarray(chunk_outs[ch]).reshape(NCORES, OB)
            sc = np.ascontiguousarray(
                a[:, CL * D:]).view(f).reshape(NCORES, CL)
            np.multiply(a[:, :CL * D].reshape(NCORES, CL, D),
                        sc[..., None], out=out[:, ch])
        list(self.pool.map(dec, range(NCH)))
        return out.reshape(B, S, D)


def kernel(**inputs) -> np.ndarray:
    global _RUNNER
    if _RUNNER is None:
        _RUNNER = _Runner()
    return _RUNNER(inputs)
